# revision 45
# baseline (speedup 1.0000x reference)
"""Depth-weighted 3x3 conv (DepthConv) Trainium2 Bass kernel (fp16).

Math (per batch element):
  sim[k, p] = exp(-|depth[p + off_k] - depth[p]|)   (9 taps, off = dh*W + dw)
  out[o, p] = sum_{c,k} W[o,c,k] * sim[k,p] * x[c, p + off_k] + bias[o]

Sharding: data-parallel over batch, one batch element per NeuronCore (8).

Layout (per core): SBUF partitions = 64 channels x {top, bottom} half.
Free dim = padded flat image, WB=164 per row ([P P x0..x159 P P]), 84
rows; all row starts EVEN so every DVE op is 4B-aligned (2x mode).  Out
pixel (j, w) center q = (j+2)*164 + 2 + w in both halves.  x2o is the
parity-shifted copy (x2o[i] = x2e[i+1]) built ON-CHIP by DVE copy so
odd-offset tap products keep the DVE 2x alignment.

Host prep (make_in_maps, pure layout transforms): x pre-padded into the
x2e layout (fp16); depth pre-padded per half in segment-aligned sub-
windows (dsb) plus host-pre-shifted bootstrap rows (dbootA/B) for
segments 0-1; weights BLOCK-DIAGONAL wt2[64h+c, t, 64h'+o] =
W[o,c,t]*(h==h')/255 (center tap unscaled) so one [128,128] lhsT drives
both image halves on the full PE array in a single matmul per tap (the
/255 undoes the u8 sim scaling below).

Device pipeline (all math on device):
  - BOOTSTRAP (segments 0-1): maps-in-free-dim rows of FD=908 -> one
    DVE sub + ACT |x| + ACT exp, so the first sim broadcast fires early
    instead of waiting for the full compact-sim pipeline.
  - compact sim (segments 2-7): two stride-paired DVE subs on the depth
    tile (map slots ordered (1,164,163,165) so sub pairs have uniform
    strides), |x| on DVE (neg+max; keeps ACT clear for the bootstrap
    exps), then ACT exp with scale=-1, bias=ln(255): half 0 to UINT8
    (= round(255*exp(-|dd|))), half 1 to fp16 (x255).
  - linearize to DRAM (s8u8 u8 / s8f16 fp16, 4 map rows in flat image
    coords), then per segment the 64x broadcast into sim_b [128,
    4*1810]: half 0 via ONE SWDGE u8->fp16 cast-DMA (halves its HBM
    read traffic), half 1 via HWDGE fp16 alternating sync/scalar queues
    so the broadcast chain is not serialized on one queue.
  - products: ONE merged 4-map prod mul (stride-0 x-repeat) + xm muls
    (slots 2,3 merged via stride-2 x2o reads) -- all DVE 2x mode.
  - 9 taps accumulate into two [128,1024] fp32 PSUMs per segment, TAP-
    OUTER order (one weight load per tap instead of four), center tap
    first (needs only x2e, so the PE opens psum groups before the DVE
    products land); ACT evacuates with fused bias, stripping row pads
    via strided PSUM read; fp16 out DMA.
  - 8 segments fully pipelined (sim broadcast prefetched 2 ahead,
    x chunk B loaded during the loop).
"""

import functools
import math
import os
import sys

import numpy as np

for _p in ("/opt/trn_rl_repo",):
    if os.path.isdir(_p) and _p not in sys.path:
        sys.path.insert(0, _p)

import concourse.bass as bass
import concourse.mybir as mybir
import concourse.tile as tile
from concourse import bacc
from concourse.bass_utils import run_bass_kernel_spmd

# ---------------------------------------------------------------- constants
B, C, H, W = 8, 64, 160, 160
O = 64
KK = 9
WB = 164                   # padded row: [P P x0..x159 P P]
NROWG = 84                 # buffer rows per half
FLATG = NROWG * WB         # 13776
DPAD_W = FLATG + 2 * WB    # depth pad width (covers shifted reads)
CCH = FLATG // 16          # 861: compact-sim column chunk
Q0 = 2 * WB + 2            # center flat index of out pixel (0,0): 330
NCORES = 8

NSEG = 8
SEGROWS = 10               # out-rows per segment per half
SEGQ = SEGROWS * WB        # 1640
HALO = 166                 # even, >= max |off| (165)
PSPAN = SEGQ + HALO        # 1806: prod tile used span
WINB = 1810                # sim window width (even); whole 4-map window
                           # is CONTIGUOUS per (half, seg) in DRAM so the
                           # 64x broadcast is one 7.2/14.5KB desc per part
SUBW = WINB // 2 + 1       # 906: compact-sim sub-window (even, x2 overlap)
TSW = 4 * SUBW             # compact sim free width per partition
DVW = SUBW + HALO + 2      # 1074: depth sub-window incl. tap halo
PIECE = WINB // 2          # 905: bootstrap piece width (8 tile a window)
BOOTW = 908                # bootstrap row width (even, >= PIECE)
NBOOT = 2                  # segments computed via the bootstrap path
CHW = 5 * WB               # 820 q-span per psum chunk
SUBS = (512, 308)          # matmul N splits at fp32 PSUM bank boundary
NCHUNK = 2                 # psum chunks per segment
XSPLIT = 44 * WB           # x load chunk boundary (segments 0-3 vs 4-7)

# (dh, dw, off) for the 4 positive-offset maps, in SLOT order chosen so
# the compact-sim subs pair with uniform strides: slots (0,1) offs (1,164)
# stride 163; slots (2,3) offs (163,165) stride 2.
MAPS = [(0, 1, 1), (1, 0, WB), (1, -1, WB - 1), (1, 1, WB + 1)]

LOG255 = float(math.log(255.0))

F32 = mybir.dt.float32
F16 = mybir.dt.float16
U8 = mybir.dt.uint8


def _tapidx(dh, dw):
    return (dh + 1) * 3 + (dw + 1)


def _build_program():
    nc = bacc.Bacc(None)
    x2e_d = nc.declare_dram_parameter("x2e", [128, FLATG], F16, isOutput=False)
    # depth in segment-aligned sub-windows: dsb[32h+8u+s, i] =
    # D_h[winbase_s + 904*u + i]  (two 906-wide sub-windows tile each
    # 1810-wide segment window; +halo for the tap offsets).  The halves
    # sit at partition bases 0/32 because ACT ops must start on a 32-
    # partition boundary (rows 16-31 and 48-63 are unused filler).
    dp_d = nc.declare_dram_parameter("dsb", [64, DVW], F16, isOutput=False)
    # bootstrap depth for segments 0-1: maps unrolled into 905-wide
    # pieces (row 32h+8s+r, r=2m+v), HOST-pre-shifted so one DVE sub
    # computes all map diffs: dbA[row,j]=D[wb+905v+j+off_m], dbB=unshifted
    dbA_d = nc.declare_dram_parameter("dbootA", [64, BOOTW], F16, isOutput=False)
    dbB_d = nc.declare_dram_parameter("dbootB", [64, BOOTW], F16, isOutput=False)
    # block-diagonal weights: wt2[64h+c, t, 64h'+o] = W[o,c,t]/255 * (h==h'),
    # so ONE [128,128] lhsT drives both halves on the full PE array
    wt_d = nc.declare_dram_parameter("wt2", [128, KK, 128], F16, isOutput=False)
    b_d = nc.declare_dram_parameter("bias2", [2 * O], F32, isOutput=False)
    out_d = nc.declare_dram_parameter("out", [O, H, W], F16, isOutput=True)

    Exp = mybir.ActivationFunctionType.Exp
    Abs = mybir.ActivationFunctionType.Abs
    Ident = mybir.ActivationFunctionType.Identity

    with tile.TileContext(nc) as tc:
        with (
            tc.tile_pool(name="dramp", bufs=1, space="DRAM") as dramp,
            tc.tile_pool(name="singles", bufs=1) as singles,
            tc.tile_pool(name="simp", bufs=3) as simp,
            tc.tile_pool(name="prodp", bufs=2) as prodp,
            tc.tile_pool(name="xmp", bufs=4) as xmp,
            tc.tile_pool(name="xm13p", bufs=2) as xm13p,
            tc.tile_pool(name="stgp", bufs=2) as stgp,
            tc.tile_pool(name="cpsum", bufs=4, space="PSUM") as cpsum,
        ):
            x2e = singles.tile([128, FLATG], F16)
            x2o = singles.tile([128, FLATG], F16)
            wt = singles.tile([128, KK, 128], F16)
            b2 = singles.tile([128, 1], F32)
            dsb = singles.tile([64, DVW], F16)
            ts32 = singles.tile([64, TSW], F16)
            tsu8 = singles.tile([32, TSW], U8)    # rows 0:16 used (h=0)
            tsf16 = singles.tile([64, TSW], F16)  # rows 32:48 used (h=1)
            ln255 = singles.tile([64, 1], F32)
            nc.vector.memset(ln255[:], LOG255)
            dbA = singles.tile([64, BOOTW], F16)
            dbB = singles.tile([64, BOOTW], F16)
            babs = singles.tile([64, BOOTW], F16)
            bu8 = singles.tile([32, BOOTW], U8)   # rows 0:16 used
            bf16 = singles.tile([64, BOOTW], F16)  # rows 32:48 used

            # ---------------- BOOTSTRAP: segments 0-1 sim via tiny
            # maps-in-free-dim ops (FD=908 not 3624) so the first
            # broadcast fires ~10us in, not ~40us.  One sub thanks to the
            # host-pre-shifted dbA/dbB rows.
            nc.sync.dma_start(out=dbA[:], in_=dbA_d[:])
            nc.scalar.dma_start(out=dbB[:], in_=dbB_d[:])
            nc.vector.tensor_sub(babs[:], dbA[:], dbB[:])
            nc.scalar.activation(out=babs[:], in_=babs[:], func=Abs)
            nc.scalar.activation(
                out=bu8[0:16, :], in_=babs[0:16, :], func=Exp,
                scale=-1.0, bias=ln255[0:16],
            )
            nc.scalar.activation(
                out=bf16[32:48, :], in_=babs[32:48, :], func=Exp,
                scale=-1.0, bias=ln255[32:48],
            )

            # ---------------- depth to SBUF once; the compact sim then
            # never touches HBM until the linearize.  ts32[p, slot*SUBW
            # + i] = D[q+off_slot] - D[q] (q = window(p) + i) via two
            # stride-paired subs directly on dsb (no fill DMAs).
            nc.sync.dma_start(out=dsb[:], in_=dp_d[:])
            t32f = ts32[:]
            dsbf = dsb[:]

            def _sub_pair(slot0, in0_off, in0_stride):
                nc.vector.tensor_sub(
                    bass.AP(
                        tensor=t32f.tensor,
                        offset=t32f.offset + slot0 * SUBW,
                        ap=[list(t32f.ap[0]), [SUBW, 2], [1, SUBW]],
                    ),
                    bass.AP(
                        tensor=dsbf.tensor,
                        offset=dsbf.offset + in0_off,
                        ap=[list(dsbf.ap[0]), [in0_stride, 2], [1, SUBW]],
                    ),
                    bass.AP(
                        tensor=dsbf.tensor,
                        offset=dsbf.offset,
                        ap=[list(dsbf.ap[0]), [0, 2], [1, SUBW]],
                    ),
                )

            _sub_pair(0, 1, WB - 1)      # slots 0,1: offs 1, 164
            _sub_pair(2, WB - 1, 2)      # slots 2,3: offs 163, 165
            # |dd| on DVE (neg 4x + max 2x) so the big main-path abs
            # can't get scheduled into the ACT queue ahead of the boot
            # exps (ACT is the bootstrap critical path)
            tneg = singles.tile([64, TSW], F16)
            nc.vector.tensor_scalar_mul(tneg[:], ts32[:], -1.0)
            nc.vector.tensor_max(ts32[:], ts32[:], tneg[:])
            # sim scaled x255: Exp(-|dd| + ln 255).  Half 0 goes to u8
            # (halves its HBM broadcast traffic; SWDGE casts back to fp16
            # on the fly), half 1 stays fp16 for the HWDGE broadcast.
            nc.scalar.activation(
                out=tsu8[0:16, :], in_=ts32[0:16, :], func=Exp,
                scale=-1.0, bias=ln255[0:16],
            )
            nc.scalar.activation(
                out=tsf16[32:48, :], in_=ts32[32:48, :], func=Exp,
                scale=-1.0, bias=ln255[32:48],
            )

            # linearize: s8x row m = map, full flat image coords (so the
            # 64x broadcast reads 4 spread-out rows, not one hot region);
            # sub-window u of segment s lands at 164 + 1640*s + 904*u
            s8u8 = dramp.tile([4, FLATG], U8)
            s8f16 = dramp.tile([4, FLATG], F16)

            def _lin_boot(dst_t, src_t, row0, s, eng):
                # bootstrap rows (m:4, v:2) of 905 -> window of segment s
                dstf = dst_t[:]
                dst = bass.AP(
                    tensor=dstf.tensor,
                    offset=dstf.offset + (Q0 - HALO) + s * SEGQ,
                    ap=[[FLATG, 4], [PIECE, 2], [1, PIECE]],
                )
                eng.dma_start(
                    out=dst,
                    in_=src_t[row0 + 8 * s : row0 + 8 * s + 8, 0:PIECE],
                )

            # all on the sync queue, s0 first: the s0 lins + broadcasts
            # are the critical path to the first products
            _lin_boot(s8u8, bu8, 0, 0, nc.sync)
            _lin_boot(s8f16, bf16, 32, 0, nc.sync)
            _lin_boot(s8u8, bu8, 0, 1, nc.sync)
            _lin_boot(s8f16, bf16, 32, 1, nc.sync)

            def _lin(dst_t, src_t, row0, eng):
                # main rows: segments NBOOT..7 only (boot covered 0-1)
                dstf = dst_t[:]
                for u in range(2):
                    dst = bass.AP(
                        tensor=dstf.tensor,
                        offset=dstf.offset + (Q0 - HALO) + NBOOT * SEGQ
                        + u * (SUBW - 2),
                        ap=[[SEGQ, 8 - NBOOT], [FLATG, 4], [1, SUBW]],
                    )
                    eng.dma_start(
                        out=dst,
                        in_=src_t[
                            row0 + 8 * u + NBOOT : row0 + 8 * u + 8, :
                        ],
                    )

            _lin(s8u8, tsu8, 0, nc.sync)
            _lin(s8f16, tsf16, 32, nc.scalar)

            # ---------------- x chunk A + weights.  Chunk B (needed from
            # segment 4) is deferred into the loop.  x2o = x2e shifted by
            # one element, built on-chip (saves a full 3.5 MB HBM load).
            # x2e rides the SWDGE queue: it is idle before the first
            # broadcast, while the sync queue must stay clear for the
            # bootstrap lins + segment-0 fp16 broadcast (critical path).
            nc.gpsimd.dma_start(out=x2e[:, 0:XSPLIT], in_=x2e_d[:, 0:XSPLIT])
            nc.scalar.dma_start(out=wt[:], in_=wt_d[:])
            nc.scalar.dma_start(
                out=b2[:], in_=b_d.rearrange("(p one) -> p one", one=1)
            )
            nc.vector.tensor_copy(
                out=x2o[:, 0 : XSPLIT - 1], in_=x2e[:, 1:XSPLIT]
            )

            # ---------------- main loop
            s8u8_f = s8u8[:]
            s8f16_f = s8f16[:]

            def emit_bcast(s):
                """Replicate segment s's 4-map sim window x64: half 0 via
                SWDGE u8->fp16 cast DMA, half 1 via HWDGE fp16."""
                winbase = Q0 + s * SEGQ - HALO
                sim_b = simp.tile([128, 4 * WINB], F16, tag="sim")
                sbv = sim_b.rearrange("p (m i) -> p m i", m=4, i=WINB)
                src0 = bass.AP(
                    tensor=s8u8_f.tensor,
                    offset=s8u8_f.offset + winbase,
                    ap=[[0, 64], [FLATG, 4], [1, WINB]],
                )
                nc.gpsimd.dma_start(out=sbv[0:64], in_=src0)
                src1 = bass.AP(
                    tensor=s8f16_f.tensor,
                    offset=s8f16_f.offset + winbase,
                    ap=[[0, 64], [FLATG, 4], [1, WINB]],
                )
                # alternate the fp16 half across the two HWDGE queues so
                # the broadcast chain isn't serialized on one of them
                eng = nc.sync if s % 2 == 0 else nc.scalar
                eng.dma_start(out=sbv[64:128], in_=src1)
                return sim_b

            sim_tiles = [emit_bcast(0), emit_bcast(1)]

            for s in range(NSEG):
                qs = Q0 + s * SEGQ
                winbase = qs - HALO

                if s == 1:
                    nc.gpsimd.dma_start(
                        out=x2e[:, XSPLIT:], in_=x2e_d[:, XSPLIT:]
                    )
                    nc.vector.tensor_copy(
                        out=x2o[:, XSPLIT - 1 : FLATG - 1],
                        in_=x2e[:, XSPLIT:FLATG],
                    )
                if s + 2 < NSEG:
                    sim_tiles.append(emit_bcast(s + 2))

                sim_b = sim_tiles[s]
                sbv = sim_b.rearrange("p (m i) -> p m i", m=4, i=WINB)

                # merged 4-map prod: in0 = x2e window repeated (stride 0)
                prod_b = prodp.tile([128, 4 * WINB], F16, tag="prod")
                pbv = prod_b.rearrange("p (m i) -> p m i", m=4, i=WINB)
                x2e_f = x2e[:]
                xrep = bass.AP(
                    tensor=x2e_f.tensor,
                    offset=x2e_f.offset + winbase,
                    ap=[list(x2e_f.ap[0]), [0, 4], [1, PSPAN]],
                )
                nc.vector.tensor_mul(
                    pbv[:, :, 0:PSPAN], xrep, sbv[:, :, 0:PSPAN]
                )

                # xm products: slots 0,1 single ops; slots 2,3 (offs
                # 163/165) merged via stride-2 x2o reads
                xm0 = xmp.tile([128, SEGQ], F16, tag="xm")
                nc.vector.tensor_mul(
                    xm0[:], x2o[:, qs : qs + SEGQ], sbv[:, 0, HALO : HALO + SEGQ]
                )
                xm1 = xmp.tile([128, SEGQ], F16, tag="xm")
                nc.vector.tensor_mul(
                    xm1[:],
                    x2e[:, qs + WB : qs + WB + SEGQ],
                    sbv[:, 1, HALO : HALO + SEGQ],
                )
                xm23 = xm13p.tile([128, 2 * SEGQ], F16, tag="xm23")
                x2o_f = x2o[:]
                sb_f = sim_b[:]
                nc.vector.tensor_mul(
                    bass.AP(
                        tensor=xm23[:].tensor,
                        offset=xm23[:].offset,
                        ap=[list(xm23[:].ap[0]), [SEGQ, 2], [1, SEGQ]],
                    ),
                    bass.AP(
                        tensor=x2o_f.tensor,
                        offset=x2o_f.offset + qs + WB - 2,
                        ap=[list(x2o_f.ap[0]), [2, 2], [1, SEGQ]],
                    ),
                    bass.AP(
                        tensor=sb_f.tensor,
                        offset=sb_f.offset + 2 * WINB + HALO,
                        ap=[list(sb_f.ap[0]), [WINB, 2], [1, SEGQ]],
                    ),
                )

                # tap sources: (weight idx, tile, base offset); actual rhs
                # window = base + j*CHW + o2.  Center tap first: it only
                # needs x2e, so the PE can open the psum groups before the
                # DVE products for this segment land.
                tapsrc = [(_tapidx(0, 0), x2e, qs)]
                for m, (dh, dw, off) in enumerate(MAPS):
                    tapsrc.append(
                        (_tapidx(-dh, -dw), prod_b, m * WINB + HALO - off)
                    )
                xms = [xm0, xm1, xm23, xm23]
                xmoff = [0, 0, 0, SEGQ]
                for m, (dh, dw, off) in enumerate(MAPS):
                    tapsrc.append((_tapidx(dh, dw), xms[m], xmoff[m]))

                # matmuls TAP-OUTER: one weight load per tap, 4 matmuls
                # (2 chunks x 2 bank-subs) with the same stationary lhsT.
                psums = []
                for _j in range(NCHUNK):
                    cps = cpsum.tile([128, 1024], F32, tag="cps")
                    psums.append(cps)
                ntap = len(tapsrc)
                for ti, (widx, rsrc, rbase) in enumerate(tapsrc):
                    for j in range(NCHUNK):
                        o2 = 0
                        for nn in SUBS:
                            roff = rbase + j * CHW + o2
                            nc.tensor.matmul(
                                psums[j][:, o2 : o2 + nn],
                                wt[:, widx, :],
                                rsrc[:, roff : roff + nn],
                                start=(ti == 0),
                                stop=(ti == ntap - 1),
                                skip_group_check=True,
                            )
                            o2 += nn

                # strip pad columns: psum rows of 164 -> 160
                stg = stgp.tile([128, SEGROWS * W], F16, tag="stg")
                for j in range(NCHUNK):
                    psum = psums[j]
                    nc.scalar.activation(
                        out=stg[:, j * 5 * W : (j + 1) * 5 * W].rearrange(
                            "p (r w) -> p r w", r=5, w=W
                        ),
                        in_=bass.AP(
                            tensor=psum[:].tensor,
                            offset=psum[:].offset,
                            ap=[list(psum[:].ap[0]), [WB, 5], [1, W]],
                        ),
                        func=Ident,
                        bias=b2[:],
                        scale=1.0,
                    )

                r0o = SEGROWS * s
                if s == NSEG - 1:
                    # final segment: flush per 5-row chunk, halves on
                    # separate queues (sync is idle by then) so the last
                    # out DMAs drain in parallel
                    for j in range(NCHUNK):
                        ra = r0o + 5 * j
                        sl = slice(j * 5 * W, (j + 1) * 5 * W)
                        nc.sync.dma_start(
                            out=out_d[:, ra : ra + 5, :].rearrange(
                                "c r w -> c (r w)"
                            ),
                            in_=stg[0:64, sl],
                        )
                        nc.scalar.dma_start(
                            out=out_d[:, 80 + ra : 80 + ra + 5, :].rearrange(
                                "c r w -> c (r w)"
                            ),
                            in_=stg[64:128, sl],
                        )
                else:
                    nc.scalar.dma_start(
                        out=out_d[:, r0o : r0o + SEGROWS, :].rearrange(
                            "c r w -> c (r w)"
                        ),
                        in_=stg[0:64, :],
                    )
                    nc.scalar.dma_start(
                        out=out_d[
                            :, 80 + r0o : 80 + r0o + SEGROWS, :
                        ].rearrange("c r w -> c (r w)"),
                        in_=stg[64:128, :],
                    )

    return nc


@functools.lru_cache(maxsize=1)
def _get_program():
    return _build_program()


def make_in_maps(x, depth, weights, bias):
    x = np.asarray(x, np.float32)
    depth = np.asarray(depth, np.float32)
    # /255 undoes the u8 sim scaling -- except the center tap, whose rhs
    # is raw x (sim == 1 exactly, never multiplied by the 255-scaled sim)
    wscale = np.full((1, 1, KK), 1.0 / 255.0)
    wscale[0, 0, (KK // 2)] = 1.0
    wbase = np.ascontiguousarray(
        weights.reshape(O, C, KK) * wscale
    ).transpose(1, 2, 0).astype(np.float16)
    wt2 = np.zeros((128, KK, 128), np.float16)
    wt2[0:64, :, 0:64] = wbase
    wt2[64:128, :, 64:128] = wbase
    b2 = np.concatenate([bias, bias]).astype(np.float32)

    n = x.shape[0]
    # padded layouts (pure layout transforms; all math stays on device)
    x2e = np.zeros((n, 128, NROWG, WB), np.float16)
    x2e[:, 0:64, 2:83, 2:162] = x[:, :, 0:81, :]
    x2e[:, 64:128, 1:82, 2:162] = x[:, :, 79:160, :]
    x2e = x2e.reshape(n, 128, FLATG)

    dpad = np.zeros((n, 2, DPAD_W), np.float16)
    dpv = dpad.reshape(n, 2, DPAD_W // WB, WB)
    dpv[:, 0, 2:83, 2:162] = depth[:, 0, 0:81, :]
    dpv[:, 1, 1:82, 2:162] = depth[:, 0, 79:160, :]
    # segment-aligned sub-window layout: dsb[32h+8u+s] covers the u-th
    # 906-wide piece (+tap halo) of segment s's 1810-wide sim window
    dsb = np.zeros((n, 64, DVW), np.float16)
    for h in range(2):
        for u in range(2):
            for s in range(NSEG):
                ws = (Q0 - HALO) + SEGQ * s + (SUBW - 2) * u
                dsb[:, 32 * h + 8 * u + s] = dpad[:, h, ws : ws + DVW]

    # bootstrap rows for segments 0..NBOOT-1: row 32h+8s+(2m+v) holds the
    # (m, v) 905-piece of segment s's window, pre-shifted by off_m in dbA
    dbA = np.zeros((n, 64, BOOTW), np.float16)
    dbB = np.zeros((n, 64, BOOTW), np.float16)
    for h in range(2):
        for s in range(NBOOT):
            for m, (_dh, _dw, off) in enumerate(MAPS):
                for v in range(2):
                    ws = (Q0 - HALO) + SEGQ * s + PIECE * v
                    row = 32 * h + 8 * s + 2 * m + v
                    dbA[:, row] = dpad[:, h, ws + off : ws + off + BOOTW]
                    dbB[:, row] = dpad[:, h, ws : ws + BOOTW]

    base = {"wt2": wt2, "bias2": b2}
    return [
        {
            "x2e": np.ascontiguousarray(x2e[i]),
            "dsb": np.ascontiguousarray(dsb[i]),
            "dbootA": np.ascontiguousarray(dbA[i]),
            "dbootB": np.ascontiguousarray(dbB[i]),
            **base,
        }
        for i in range(n)
    ]


def kernel(x, depth, weights, bias):
    nc = _get_program()
    if not nc.is_finalized():
        nc.finalize()
    in_maps = make_in_maps(x, depth, weights, bias)
    res = run_bass_kernel_spmd(nc, in_maps, list(range(NCORES)))
    out = np.stack([np.asarray(res.results[i]["out"]) for i in range(NCORES)])
    return out.astype(np.float32)


# revision 46
# speedup vs baseline: 1.0196x; 1.0196x over previous
"""Depth-weighted 3x3 conv (DepthConv) Trainium2 Bass kernel (fp16).

Math (per batch element):
  sim[k, p] = exp(-|depth[p + off_k] - depth[p]|)   (9 taps, off = dh*W + dw)
  out[o, p] = sum_{c,k} W[o,c,k] * sim[k,p] * x[c, p + off_k] + bias[o]

Sharding: data-parallel over batch, one batch element per NeuronCore (8).

Layout (per core): SBUF partitions = 64 channels x {top, bottom} half.
Free dim = padded flat image, WB=164 per row ([P P x0..x159 P P]), 84
rows; all row starts EVEN so every DVE op is 4B-aligned (2x mode).  Out
pixel (j, w) center q = (j+2)*164 + 2 + w in both halves.  x2o is the
parity-shifted copy (x2o[i] = x2e[i+1]) built ON-CHIP by DVE copy so
odd-offset tap products keep the DVE 2x alignment.

Host prep (make_in_maps, pure layout transforms): x pre-padded into the
x2e layout (fp16); depth pre-padded per half in segment-aligned sub-
windows (dsb) plus host-pre-shifted bootstrap rows (dbootA/B) for
segments 0-1; weights BLOCK-DIAGONAL wt2[64h+c, t, 64h'+o] =
W[o,c,t]*(h==h')/255 (center tap unscaled) so one [128,128] lhsT drives
both image halves on the full PE array in a single matmul per tap (the
/255 undoes the u8 sim scaling below).

Device pipeline (all math on device):
  - BOOTSTRAP (segments 0-1): maps-in-free-dim rows of FD=908 -> one
    DVE sub + ACT |x| + ACT exp, so the first sim broadcast fires early
    instead of waiting for the full compact-sim pipeline.
  - compact sim (segments 2-7): two stride-paired DVE subs on the depth
    tile (map slots ordered (1,164,163,165) so sub pairs have uniform
    strides), |x| on DVE (neg+max; keeps ACT clear for the bootstrap
    exps), then ACT exp with scale=-1, bias=ln(255): half 0 to UINT8
    (= round(255*exp(-|dd|))), half 1 to fp16 (x255).
  - linearize to DRAM (s8u8 u8 / s8f16 fp16, 4 map rows in flat image
    coords), then per segment the 64x broadcast into sim_b [128,
    4*1810]: half 0 via ONE SWDGE u8->fp16 cast-DMA (halves its HBM
    read traffic), half 1 via HWDGE fp16 alternating sync/scalar queues
    so the broadcast chain is not serialized on one queue.
  - products: ONE merged 4-map prod mul (stride-0 x-repeat) + xm muls
    (slots 2,3 merged via stride-2 x2o reads) -- all DVE 2x mode.
  - 9 taps accumulate into two [128,1024] fp32 PSUMs per segment, TAP-
    OUTER order (one weight load per tap instead of four), center tap
    first (needs only x2e, so the PE opens psum groups before the DVE
    products land); ACT evacuates with fused bias, stripping row pads
    via strided PSUM read; fp16 out DMA.
  - 8 segments fully pipelined (sim broadcast prefetched 2 ahead,
    x chunk B loaded during the loop).
"""

import functools
import math
import os
import sys

import numpy as np

for _p in ("/opt/trn_rl_repo",):
    if os.path.isdir(_p) and _p not in sys.path:
        sys.path.insert(0, _p)

import concourse.bass as bass
import concourse.mybir as mybir
import concourse.tile as tile
from concourse import bacc
from concourse.bass_utils import run_bass_kernel_spmd

# ---------------------------------------------------------------- constants
B, C, H, W = 8, 64, 160, 160
O = 64
KK = 9
WB = 164                   # padded row: [P P x0..x159 P P]
NROWG = 84                 # buffer rows per half
FLATG = NROWG * WB         # 13776
DPAD_W = FLATG + 2 * WB    # depth pad width (covers shifted reads)
CCH = FLATG // 16          # 861: compact-sim column chunk
Q0 = 2 * WB + 2            # center flat index of out pixel (0,0): 330
NCORES = 8

NSEG = 8
SEGROWS = 10               # out-rows per segment per half
SEGQ = SEGROWS * WB        # 1640
HALO = 166                 # even, >= max |off| (165)
PSPAN = SEGQ + HALO        # 1806: prod tile used span
WINB = 1810                # sim window width (even); whole 4-map window
                           # is CONTIGUOUS per (half, seg) in DRAM so the
                           # 64x broadcast is one 7.2/14.5KB desc per part
SUBW = WINB // 2 + 1       # 906: compact-sim sub-window (even, x2 overlap)
TSW = 4 * SUBW             # compact sim free width per partition
DVW = SUBW + HALO + 2      # 1074: depth sub-window incl. tap halo
PIECE = WINB // 2          # 905: bootstrap piece width (8 tile a window)
BOOTW = 908                # bootstrap row width (even, >= PIECE)
NBOOT = 2                  # segments computed via the bootstrap path
CHW = 5 * WB               # 820 q-span per psum chunk
SUBS = (512, 308)          # matmul N splits at fp32 PSUM bank boundary
NCHUNK = 2                 # psum chunks per segment
XSPLIT = 44 * WB           # x load chunk boundary (segments 0-3 vs 4-7)

# (dh, dw, off) for the 4 positive-offset maps, in SLOT order chosen so
# the compact-sim subs pair with uniform strides: slots (0,1) offs (1,164)
# stride 163; slots (2,3) offs (163,165) stride 2.
MAPS = [(0, 1, 1), (1, 0, WB), (1, -1, WB - 1), (1, 1, WB + 1)]

LOG255 = float(math.log(255.0))

F32 = mybir.dt.float32
F16 = mybir.dt.float16
U8 = mybir.dt.uint8


def _tapidx(dh, dw):
    return (dh + 1) * 3 + (dw + 1)


def _build_program():
    nc = bacc.Bacc(None)
    x2e_d = nc.declare_dram_parameter("x2e", [128, FLATG], F16, isOutput=False)
    # depth in segment-aligned sub-windows: dsb[32h+8u+s, i] =
    # D_h[winbase_s + 904*u + i]  (two 906-wide sub-windows tile each
    # 1810-wide segment window; +halo for the tap offsets).  The halves
    # sit at partition bases 0/32 because ACT ops must start on a 32-
    # partition boundary (rows 16-31 and 48-63 are unused filler).
    dp_d = nc.declare_dram_parameter("dsb", [64, DVW], F16, isOutput=False)
    # bootstrap depth for segments 0-1: maps unrolled into 905-wide
    # pieces (row 32h+8s+r, r=2m+v), HOST-pre-shifted so one DVE sub
    # computes all map diffs: dbA[row,j]=D[wb+905v+j+off_m], dbB=unshifted
    dbA_d = nc.declare_dram_parameter("dbootA", [64, BOOTW], F16, isOutput=False)
    dbB_d = nc.declare_dram_parameter("dbootB", [64, BOOTW], F16, isOutput=False)
    # block-diagonal weights: wt2[64h+c, t, 64h'+o] = W[o,c,t]/255 * (h==h'),
    # so ONE [128,128] lhsT drives both halves on the full PE array
    wt_d = nc.declare_dram_parameter("wt2", [128, KK, 128], F16, isOutput=False)
    b_d = nc.declare_dram_parameter("bias2", [2 * O], F32, isOutput=False)
    out_d = nc.declare_dram_parameter("out", [O, H, W], F16, isOutput=True)

    Exp = mybir.ActivationFunctionType.Exp
    Abs = mybir.ActivationFunctionType.Abs
    Ident = mybir.ActivationFunctionType.Identity

    with tile.TileContext(nc) as tc:
        with (
            tc.tile_pool(name="dramp", bufs=1, space="DRAM") as dramp,
            tc.tile_pool(name="singles", bufs=1) as singles,
            tc.tile_pool(name="simp", bufs=3) as simp,
            tc.tile_pool(name="prodp", bufs=2) as prodp,
            tc.tile_pool(name="xmp", bufs=4) as xmp,
            tc.tile_pool(name="xm13p", bufs=2) as xm13p,
            tc.tile_pool(name="stgp", bufs=2) as stgp,
            tc.tile_pool(name="cpsum", bufs=4, space="PSUM") as cpsum,
        ):
            x2e = singles.tile([128, FLATG], F16)
            x2o = singles.tile([128, FLATG], F16)
            wt = singles.tile([128, KK, 128], F16)
            b2 = singles.tile([128, 1], F32)
            dsb = singles.tile([64, DVW], F16)
            ts32 = singles.tile([64, TSW], F16)
            tsu8 = singles.tile([32, TSW], U8)    # rows 0:16 used (h=0)
            tsf16 = singles.tile([64, TSW], F16)  # rows 32:48 used (h=1)
            ln255 = singles.tile([64, 1], F32)
            nc.vector.memset(ln255[:], LOG255)
            dbA = singles.tile([64, BOOTW], F16)
            dbB = singles.tile([64, BOOTW], F16)
            babs = singles.tile([64, BOOTW], F16)
            bu8 = singles.tile([32, BOOTW], U8)   # rows 0:16 used
            bf16 = singles.tile([64, BOOTW], F16)  # rows 32:48 used

            # ---------------- BOOTSTRAP: segments 0-1 sim via tiny
            # maps-in-free-dim ops (FD=908 not 3624) so the first
            # broadcast fires ~10us in, not ~40us.  One sub thanks to the
            # host-pre-shifted dbA/dbB rows.
            nc.sync.dma_start(out=dbA[:], in_=dbA_d[:])
            nc.scalar.dma_start(out=dbB[:], in_=dbB_d[:])
            nc.vector.tensor_sub(babs[:], dbA[:], dbB[:])
            nc.scalar.activation(out=babs[:], in_=babs[:], func=Abs)
            nc.scalar.activation(
                out=bu8[0:16, :], in_=babs[0:16, :], func=Exp,
                scale=-1.0, bias=ln255[0:16],
            )
            nc.scalar.activation(
                out=bf16[32:48, :], in_=babs[32:48, :], func=Exp,
                scale=-1.0, bias=ln255[32:48],
            )

            # ---------------- depth to SBUF once; the compact sim then
            # never touches HBM until the linearize.  ts32[p, slot*SUBW
            # + i] = D[q+off_slot] - D[q] (q = window(p) + i) via two
            # stride-paired subs directly on dsb (no fill DMAs).
            nc.sync.dma_start(out=dsb[:], in_=dp_d[:])
            t32f = ts32[:]
            dsbf = dsb[:]

            def _sub_pair(slot0, in0_off, in0_stride):
                nc.vector.tensor_sub(
                    bass.AP(
                        tensor=t32f.tensor,
                        offset=t32f.offset + slot0 * SUBW,
                        ap=[list(t32f.ap[0]), [SUBW, 2], [1, SUBW]],
                    ),
                    bass.AP(
                        tensor=dsbf.tensor,
                        offset=dsbf.offset + in0_off,
                        ap=[list(dsbf.ap[0]), [in0_stride, 2], [1, SUBW]],
                    ),
                    bass.AP(
                        tensor=dsbf.tensor,
                        offset=dsbf.offset,
                        ap=[list(dsbf.ap[0]), [0, 2], [1, SUBW]],
                    ),
                )

            _sub_pair(0, 1, WB - 1)      # slots 0,1: offs 1, 164
            _sub_pair(2, WB - 1, 2)      # slots 2,3: offs 163, 165
            # |dd| on DVE (neg 4x + max 2x) so the big main-path abs
            # can't get scheduled into the ACT queue ahead of the boot
            # exps (ACT is the bootstrap critical path)
            tneg = singles.tile([64, TSW], F16)
            nc.vector.tensor_scalar_mul(tneg[:], ts32[:], -1.0)
            nc.vector.tensor_max(ts32[:], ts32[:], tneg[:])
            # sim scaled x255: Exp(-|dd| + ln 255).  Half 0 goes to u8
            # (halves its HBM broadcast traffic; SWDGE casts back to fp16
            # on the fly), half 1 stays fp16 for the HWDGE broadcast.
            nc.scalar.activation(
                out=tsu8[0:16, :], in_=ts32[0:16, :], func=Exp,
                scale=-1.0, bias=ln255[0:16],
            )
            nc.scalar.activation(
                out=tsf16[32:48, :], in_=ts32[32:48, :], func=Exp,
                scale=-1.0, bias=ln255[32:48],
            )

            # linearize: s8x row m = map, full flat image coords (so the
            # 64x broadcast reads 4 spread-out rows, not one hot region);
            # sub-window u of segment s lands at 164 + 1640*s + 904*u
            s8u8 = dramp.tile([4, FLATG], U8)
            s8f16 = dramp.tile([4, FLATG], F16)

            def _lin_boot(dst_t, src_t, row0, s, eng):
                # bootstrap rows (m:4, v:2) of 905 -> window of segment s
                dstf = dst_t[:]
                dst = bass.AP(
                    tensor=dstf.tensor,
                    offset=dstf.offset + (Q0 - HALO) + s * SEGQ,
                    ap=[[FLATG, 4], [PIECE, 2], [1, PIECE]],
                )
                eng.dma_start(
                    out=dst,
                    in_=src_t[row0 + 8 * s : row0 + 8 * s + 8, 0:PIECE],
                )

            # all on the sync queue, s0 first: the s0 lins + broadcasts
            # are the critical path to the first products
            _lin_boot(s8u8, bu8, 0, 0, nc.sync)
            _lin_boot(s8f16, bf16, 32, 0, nc.sync)
            _lin_boot(s8u8, bu8, 0, 1, nc.sync)
            _lin_boot(s8f16, bf16, 32, 1, nc.sync)

            def _lin(dst_t, src_t, row0, eng):
                # main rows: segments NBOOT..7 only (boot covered 0-1)
                dstf = dst_t[:]
                for u in range(2):
                    dst = bass.AP(
                        tensor=dstf.tensor,
                        offset=dstf.offset + (Q0 - HALO) + NBOOT * SEGQ
                        + u * (SUBW - 2),
                        ap=[[SEGQ, 8 - NBOOT], [FLATG, 4], [1, SUBW]],
                    )
                    eng.dma_start(
                        out=dst,
                        in_=src_t[
                            row0 + 8 * u + NBOOT : row0 + 8 * u + 8, :
                        ],
                    )

            _lin(s8u8, tsu8, 0, nc.sync)
            _lin(s8f16, tsf16, 32, nc.scalar)

            # ---------------- x chunk A + weights.  Chunk B (needed from
            # segment 4) is deferred into the loop.  x2o = x2e shifted by
            # one element, built on-chip (saves a full 3.5 MB HBM load).
            nc.sync.dma_start(out=x2e[:, 0:XSPLIT], in_=x2e_d[:, 0:XSPLIT])
            nc.scalar.dma_start(out=wt[:], in_=wt_d[:])
            nc.scalar.dma_start(
                out=b2[:], in_=b_d.rearrange("(p one) -> p one", one=1)
            )
            nc.vector.tensor_copy(
                out=x2o[:, 0 : XSPLIT - 1], in_=x2e[:, 1:XSPLIT]
            )

            # ---------------- main loop
            s8u8_f = s8u8[:]
            s8f16_f = s8f16[:]

            def emit_bcast(s):
                """Replicate segment s's 4-map sim window x64: half 0 via
                SWDGE u8->fp16 cast DMA, half 1 via HWDGE fp16."""
                winbase = Q0 + s * SEGQ - HALO
                sim_b = simp.tile([128, 4 * WINB], F16, tag="sim")
                sbv = sim_b.rearrange("p (m i) -> p m i", m=4, i=WINB)
                src0 = bass.AP(
                    tensor=s8u8_f.tensor,
                    offset=s8u8_f.offset + winbase,
                    ap=[[0, 64], [FLATG, 4], [1, WINB]],
                )
                nc.gpsimd.dma_start(out=sbv[0:64], in_=src0)
                src1 = bass.AP(
                    tensor=s8f16_f.tensor,
                    offset=s8f16_f.offset + winbase,
                    ap=[[0, 64], [FLATG, 4], [1, WINB]],
                )
                # alternate the fp16 half across the two HWDGE queues so
                # the broadcast chain isn't serialized on one of them
                eng = nc.sync if s % 2 == 0 else nc.scalar
                eng.dma_start(out=sbv[64:128], in_=src1)
                return sim_b

            sim_tiles = [emit_bcast(0), emit_bcast(1)]

            for s in range(NSEG):
                qs = Q0 + s * SEGQ
                winbase = qs - HALO

                if s == 1:
                    nc.sync.dma_start(
                        out=x2e[:, XSPLIT:], in_=x2e_d[:, XSPLIT:]
                    )
                    nc.vector.tensor_copy(
                        out=x2o[:, XSPLIT - 1 : FLATG - 1],
                        in_=x2e[:, XSPLIT:FLATG],
                    )
                if s + 2 < NSEG:
                    sim_tiles.append(emit_bcast(s + 2))

                sim_b = sim_tiles[s]
                sbv = sim_b.rearrange("p (m i) -> p m i", m=4, i=WINB)

                # merged 4-map prod: in0 = x2e window repeated (stride 0)
                prod_b = prodp.tile([128, 4 * WINB], F16, tag="prod")
                pbv = prod_b.rearrange("p (m i) -> p m i", m=4, i=WINB)
                x2e_f = x2e[:]
                xrep = bass.AP(
                    tensor=x2e_f.tensor,
                    offset=x2e_f.offset + winbase,
                    ap=[list(x2e_f.ap[0]), [0, 4], [1, PSPAN]],
                )
                nc.vector.tensor_mul(
                    pbv[:, :, 0:PSPAN], xrep, sbv[:, :, 0:PSPAN]
                )

                # xm products: slots 0,1 single ops; slots 2,3 (offs
                # 163/165) merged via stride-2 x2o reads
                xm0 = xmp.tile([128, SEGQ], F16, tag="xm")
                nc.vector.tensor_mul(
                    xm0[:], x2o[:, qs : qs + SEGQ], sbv[:, 0, HALO : HALO + SEGQ]
                )
                xm1 = xmp.tile([128, SEGQ], F16, tag="xm")
                nc.vector.tensor_mul(
                    xm1[:],
                    x2e[:, qs + WB : qs + WB + SEGQ],
                    sbv[:, 1, HALO : HALO + SEGQ],
                )
                xm23 = xm13p.tile([128, 2 * SEGQ], F16, tag="xm23")
                x2o_f = x2o[:]
                sb_f = sim_b[:]
                nc.vector.tensor_mul(
                    bass.AP(
                        tensor=xm23[:].tensor,
                        offset=xm23[:].offset,
                        ap=[list(xm23[:].ap[0]), [SEGQ, 2], [1, SEGQ]],
                    ),
                    bass.AP(
                        tensor=x2o_f.tensor,
                        offset=x2o_f.offset + qs + WB - 2,
                        ap=[list(x2o_f.ap[0]), [2, 2], [1, SEGQ]],
                    ),
                    bass.AP(
                        tensor=sb_f.tensor,
                        offset=sb_f.offset + 2 * WINB + HALO,
                        ap=[list(sb_f.ap[0]), [WINB, 2], [1, SEGQ]],
                    ),
                )

                # tap sources: (weight idx, tile, base offset); actual rhs
                # window = base + j*CHW + o2.  Center tap first: it only
                # needs x2e, so the PE can open the psum groups before the
                # DVE products for this segment land.
                tapsrc = [(_tapidx(0, 0), x2e, qs)]
                for m, (dh, dw, off) in enumerate(MAPS):
                    tapsrc.append(
                        (_tapidx(-dh, -dw), prod_b, m * WINB + HALO - off)
                    )
                xms = [xm0, xm1, xm23, xm23]
                xmoff = [0, 0, 0, SEGQ]
                for m, (dh, dw, off) in enumerate(MAPS):
                    tapsrc.append((_tapidx(dh, dw), xms[m], xmoff[m]))

                # matmuls TAP-OUTER: one weight load per tap, 4 matmuls
                # (2 chunks x 2 bank-subs) with the same stationary lhsT.
                psums = []
                for _j in range(NCHUNK):
                    cps = cpsum.tile([128, 1024], F32, tag="cps")
                    psums.append(cps)
                ntap = len(tapsrc)
                for ti, (widx, rsrc, rbase) in enumerate(tapsrc):
                    for j in range(NCHUNK):
                        o2 = 0
                        for nn in SUBS:
                            roff = rbase + j * CHW + o2
                            nc.tensor.matmul(
                                psums[j][:, o2 : o2 + nn],
                                wt[:, widx, :],
                                rsrc[:, roff : roff + nn],
                                start=(ti == 0),
                                stop=(ti == ntap - 1),
                                skip_group_check=True,
                            )
                            o2 += nn

                # strip pad columns: psum rows of 164 -> 160
                stg = stgp.tile([128, SEGROWS * W], F16, tag="stg")
                for j in range(NCHUNK):
                    psum = psums[j]
                    nc.scalar.activation(
                        out=stg[:, j * 5 * W : (j + 1) * 5 * W].rearrange(
                            "p (r w) -> p r w", r=5, w=W
                        ),
                        in_=bass.AP(
                            tensor=psum[:].tensor,
                            offset=psum[:].offset,
                            ap=[list(psum[:].ap[0]), [WB, 5], [1, W]],
                        ),
                        func=Ident,
                        bias=b2[:],
                        scale=1.0,
                    )

                r0o = SEGROWS * s
                if s == NSEG - 1:
                    # final segment: flush per 5-row chunk so the last out
                    # DMA starts right after the last evacuation
                    for j in range(NCHUNK):
                        ra = r0o + 5 * j
                        sl = slice(j * 5 * W, (j + 1) * 5 * W)
                        nc.scalar.dma_start(
                            out=out_d[:, ra : ra + 5, :].rearrange(
                                "c r w -> c (r w)"
                            ),
                            in_=stg[0:64, sl],
                        )
                        nc.scalar.dma_start(
                            out=out_d[:, 80 + ra : 80 + ra + 5, :].rearrange(
                                "c r w -> c (r w)"
                            ),
                            in_=stg[64:128, sl],
                        )
                else:
                    nc.scalar.dma_start(
                        out=out_d[:, r0o : r0o + SEGROWS, :].rearrange(
                            "c r w -> c (r w)"
                        ),
                        in_=stg[0:64, :],
                    )
                    nc.scalar.dma_start(
                        out=out_d[
                            :, 80 + r0o : 80 + r0o + SEGROWS, :
                        ].rearrange("c r w -> c (r w)"),
                        in_=stg[64:128, :],
                    )

    return nc


@functools.lru_cache(maxsize=1)
def _get_program():
    return _build_program()


def make_in_maps(x, depth, weights, bias):
    x = np.asarray(x, np.float32)
    depth = np.asarray(depth, np.float32)
    # /255 undoes the u8 sim scaling -- except the center tap, whose rhs
    # is raw x (sim == 1 exactly, never multiplied by the 255-scaled sim)
    wscale = np.full((1, 1, KK), 1.0 / 255.0)
    wscale[0, 0, (KK // 2)] = 1.0
    wbase = np.ascontiguousarray(
        weights.reshape(O, C, KK) * wscale
    ).transpose(1, 2, 0).astype(np.float16)
    wt2 = np.zeros((128, KK, 128), np.float16)
    wt2[0:64, :, 0:64] = wbase
    wt2[64:128, :, 64:128] = wbase
    b2 = np.concatenate([bias, bias]).astype(np.float32)

    n = x.shape[0]
    # padded layouts (pure layout transforms; all math stays on device)
    x2e = np.zeros((n, 128, NROWG, WB), np.float16)
    x2e[:, 0:64, 2:83, 2:162] = x[:, :, 0:81, :]
    x2e[:, 64:128, 1:82, 2:162] = x[:, :, 79:160, :]
    x2e = x2e.reshape(n, 128, FLATG)

    dpad = np.zeros((n, 2, DPAD_W), np.float16)
    dpv = dpad.reshape(n, 2, DPAD_W // WB, WB)
    dpv[:, 0, 2:83, 2:162] = depth[:, 0, 0:81, :]
    dpv[:, 1, 1:82, 2:162] = depth[:, 0, 79:160, :]
    # segment-aligned sub-window layout: dsb[32h+8u+s] covers the u-th
    # 906-wide piece (+tap halo) of segment s's 1810-wide sim window
    dsb = np.zeros((n, 64, DVW), np.float16)
    for h in range(2):
        for u in range(2):
            for s in range(NSEG):
                ws = (Q0 - HALO) + SEGQ * s + (SUBW - 2) * u
                dsb[:, 32 * h + 8 * u + s] = dpad[:, h, ws : ws + DVW]

    # bootstrap rows for segments 0..NBOOT-1: row 32h+8s+(2m+v) holds the
    # (m, v) 905-piece of segment s's window, pre-shifted by off_m in dbA
    dbA = np.zeros((n, 64, BOOTW), np.float16)
    dbB = np.zeros((n, 64, BOOTW), np.float16)
    for h in range(2):
        for s in range(NBOOT):
            for m, (_dh, _dw, off) in enumerate(MAPS):
                for v in range(2):
                    ws = (Q0 - HALO) + SEGQ * s + PIECE * v
                    row = 32 * h + 8 * s + 2 * m + v
                    dbA[:, row] = dpad[:, h, ws + off : ws + off + BOOTW]
                    dbB[:, row] = dpad[:, h, ws : ws + BOOTW]

    base = {"wt2": wt2, "bias2": b2}
    return [
        {
            "x2e": np.ascontiguousarray(x2e[i]),
            "dsb": np.ascontiguousarray(dsb[i]),
            "dbootA": np.ascontiguousarray(dbA[i]),
            "dbootB": np.ascontiguousarray(dbB[i]),
            **base,
        }
        for i in range(n)
    ]


def kernel(x, depth, weights, bias):
    nc = _get_program()
    if not nc.is_finalized():
        nc.finalize()
    in_maps = make_in_maps(x, depth, weights, bias)
    res = run_bass_kernel_spmd(nc, in_maps, list(range(NCORES)))
    out = np.stack([np.asarray(res.results[i]["out"]) for i in range(NCORES)])
    return out.astype(np.float32)
